# revision 20
# baseline (speedup 1.0000x reference)
"""DGL-life GCN classifier on 8 Trainium2 NeuronCores (Bass/Tile).

Strategy: shard the batched graph by dst-node across 8 cores (cuts aligned to
graph boundaries).  The axon tunnel to the devices moves ~45 MB/s, so the
wire format is minimized: each core receives only its local fp8 feature
shard (the full table is built on-device with an AllGather), edge metadata is
packed one int32 word per edge slot (gather-row index | one-hot lane << 19),
and all weights ride in a single fp32 blob.  Local nodes are permuted
(serpentine deal by in-degree) so each 512-dst block needs exactly 64 gather
columns.  Per layer: per-edge rows are fetched with indirect DMA gathers,
segment-summed into 512-dst blocks via one-hot matmuls accumulated in PSUM,
then the dense GraphConv / residual transforms run in the transposed domain
where the per-output-feature bias rides the ACT relu for free.  Feature
shards are exchanged between layers with an AllGather.  SumPooling reuses
the same one-hot matmul machinery over graph ids, followed by the 2-layer
MLP classifier.
"""
import sys
sys.path.insert(0, "/opt/trn_rl_repo")

import numpy as np
import ml_dtypes

bf16 = ml_dtypes.bfloat16
fp8 = ml_dtypes.float8_e4m3

N_NODES = 500000
N_EDGES = 8000000
N_GRAPHS = 16384
IN_F = 74
HID = 64
CLS_H = 128
N_CLASSES = 2

NCORES = 8
NPAD = 62976          # padded nodes per shard (123 blocks of 512)
NB2 = 123             # 512-node dst blocks per shard (one For_i group each)
BW = 512              # dst-block width (one-hot lane count)
COLS2 = 64            # gather columns per block (8192 edge slots, balanced)
GRP = 4               # 128-row sub-tiles per block (residual transposes)
NTOT = NCORES * NPAD  # 503808 padded global rows
GPAD = 2176           # padded graphs per shard (17 blocks of 128)
NGB = GPAD // 128     # 17
PT = 33               # pooling node tiles per graph block
ETOT = NB2 * COLS2    # edge-meta columns (7872)
PTOT = NGB * PT       # pool-meta columns (561)
WTOT8 = None          # set below: per-core slice of the padded weight blob
# invalid edge slots gather a guaranteed-pad table row (every 512-bin holds
# <=509 real nodes, so position 509 is pad everywhere) into a pad lane (511);
# pad lanes never feed real rows, so garbage stays quarantined.  This keeps
# the per-slot payload at 28 bits: uint16 low half + 12 high bits, with the
# high bits of each slot pair packed into 3 bytes.
INVALID_E = 509 | (511 << 19)
INVALID_P = 255 << 16  # pool lane 255 never matches iota 0..127
EHI = ETOT * 3 // 2    # packed high-byte columns for edge meta (11808)

# weight blob layout: (name, rows, cols); m>1 entries become bf16 tiles
WSPECS = [
    ("gW0", IN_F, HID), ("rW0", IN_F, HID),
    ("gW1", HID, HID), ("rW1", HID, HID),
    ("gW2", HID, HID), ("rW2", HID, HID),
    ("cW1", HID, CLS_H), ("cW2", CLS_H, N_CLASSES),
    ("gb0", HID, 1), ("rb0", HID, 1), ("gb1", HID, 1), ("rb1", HID, 1),
    ("gb2", HID, 1), ("rb2", HID, 1), ("cb1", CLS_H, 1), ("cb2", N_CLASSES, 1),
]
WTOT = sum(k * m for _, k, m in WSPECS)
WPAD = (WTOT + NCORES - 1) // NCORES * NCORES
WTOT8 = WPAD // NCORES

_cache = {}


def _host_prep(node_feats, src, dst, graph_ids):
    gid = np.asarray(graph_ids)
    node_of_graph_start = np.searchsorted(gid, np.arange(N_GRAPHS))
    cuts = [0]
    for k in range(1, NCORES):
        target = k * N_NODES // NCORES
        gi = np.searchsorted(node_of_graph_start, target)
        cand = [node_of_graph_start[min(gi, N_GRAPHS - 1)],
                node_of_graph_start[max(gi - 1, 0)]]
        cuts.append(int(min(cand, key=lambda x: abs(x - target))))
    cuts.append(N_NODES)
    cuts = np.asarray(cuts, np.int64)
    assert np.diff(cuts).max() <= NPAD

    gstart = [int(gid[c]) if c < N_NODES else N_GRAPHS for c in cuts[:-1]] + [N_GRAPHS]
    gstart = np.asarray(gstart, np.int64)
    assert np.diff(gstart).max() <= GPAD

    src = np.asarray(src).astype(np.int64)
    dst = np.asarray(dst).astype(np.int64)
    shard_of_dst = np.searchsorted(cuts, dst, side="right") - 1

    # pass 1: per-shard node permutation balancing edge load over 512-blocks
    # (serpentine deal of nodes sorted by in-degree)
    perms, masks = [], []
    perm_glob = np.empty(N_NODES, np.int64)
    for k in range(NCORES):
        base, n = cuts[k], cuts[k + 1] - cuts[k]
        m = shard_of_dst == k
        masks.append(m)
        indeg = np.bincount(dst[m] - base, minlength=n)
        order = np.argsort(-indeg, kind="stable")
        ids = np.arange(n)
        r, i = ids // NB2, ids % NB2
        bin_ = np.where(r % 2 == 0, i, NB2 - 1 - i)
        perm = np.empty(n, np.int64)
        perm[order] = bin_ * BW + r
        perms.append(perm)
        perm_glob[base:base + n] = k * NPAD + perm

    src_pad = perm_glob[src]

    per_core = []
    for k in range(NCORES):
        base, n = cuts[k], cuts[k + 1] - cuts[k]
        m, perm = masks[k], perms[k]
        e_src = src_pad[m]
        e_dst = perm[dst[m] - base]
        order = np.argsort(e_dst, kind="stable")
        e_src, e_dst = e_src[order], e_dst[order]
        blk = e_dst >> 9
        cnt = np.bincount(blk, minlength=NB2)
        assert cnt.max() <= COLS2 * 128, cnt.max()
        cum = np.concatenate([[0], np.cumsum(cnt)])
        slot = np.arange(len(e_dst)) - cum[blk]
        eword = np.full((NB2, COLS2 * 128), INVALID_E, np.int32)
        eword[blk, slot] = (e_src | ((e_dst & (BW - 1)) << 19)).astype(np.int32)
        eword = (eword.reshape(NB2, COLS2, 128)
                 .transpose(2, 0, 1).reshape(128, ETOT))
        elo = (eword & 0xFFFF).astype(np.uint16)
        hi12 = (eword >> 16).astype(np.int32)          # 12 bits per slot
        even, odd = hi12[:, 0::2], hi12[:, 1::2]
        ehi = np.stack([even & 0xFF,
                        (even >> 8) | ((odd & 0xF) << 4),
                        odd >> 4], axis=-1).astype(np.uint8).reshape(128, EHI)

        # pooling: local nodes sorted by graph; graph-block-aligned slots
        gl = gid[base:base + n] - gstart[k]          # local graph id per node
        gb = gl >> 7
        pcnt = np.bincount(gb, minlength=NGB)
        assert pcnt.max() <= PT * 128
        pcum = np.concatenate([[0], np.cumsum(pcnt)])
        pslot = np.arange(n) - pcum[gb]
        pword = np.full((NGB, PT * 128), INVALID_P, np.int32)
        pword[gb, pslot] = (perm | ((gl & 127) << 16)).astype(np.int32)
        pword = (pword.reshape(NGB, PT, 128)
                 .transpose(2, 0, 1).reshape(128, PTOT))
        plo = (pword & 0xFFFF).astype(np.uint16)
        phi = (pword >> 16).astype(np.uint8)

        lo16 = np.concatenate([elo, plo], axis=1)
        hi8 = np.concatenate([ehi, phi], axis=1)

        h0 = np.zeros((NPAD, IN_F), fp8)
        h0[perm] = node_feats[base:base + n].astype(fp8)

        per_core.append(dict(lo16=np.ascontiguousarray(lo16),
                             hi8=np.ascontiguousarray(hi8), h0=h0))
    return cuts, gstart, per_core


def _build_nc():
    import concourse.bass as bass
    from concourse import bacc
    import concourse.mybir as mybir
    import concourse.tile as tile

    fp32 = mybir.dt.float32
    b16 = mybir.dt.bfloat16
    f8 = mybir.dt.float8e4
    i32 = mybir.dt.int32

    nc = bacc.Bacc("TRN2", target_bir_lowering=False, debug=False,
                   num_devices=NCORES)

    h0_in = nc.dram_tensor("h0", [NPAD, IN_F], f8, kind="ExternalInput")
    lo16_in = nc.dram_tensor("lo16", [128, ETOT + PTOT], mybir.dt.uint16, kind="ExternalInput")
    hi8_in = nc.dram_tensor("hi8", [128, EHI + PTOT], mybir.dt.uint8, kind="ExternalInput")
    wpart_in = nc.dram_tensor("wpart", [WTOT8], fp32, kind="ExternalInput")
    logits_out = nc.dram_tensor("logitsT", [N_CLASSES, GPAD], fp32, kind="ExternalOutput")

    elo_ap = lo16_in[:, 0:ETOT].rearrange("p (g c) -> p g c", c=COLS2)
    ehi_ap = hi8_in[:, 0:EHI].rearrange("p (g c) -> p g c", c=COLS2 * 3 // 2)
    plo_ap = lo16_in[:, ETOT:ETOT + PTOT].rearrange("p (g c) -> p g c", c=PT)
    phi_ap = hi8_in[:, EHI:EHI + PTOT].rearrange("p (g c) -> p g c", c=PT)
    # L0 residual rhs source: [g, b, 128, f] view of the local shard
    h0r_ap = h0_in[:].rearrange("(g b p) f -> g b p f", b=GRP, p=128)

    Relu = mybir.ActivationFunctionType.Relu
    EQ = mybir.AluOpType.is_equal
    AND = mybir.AluOpType.bitwise_and
    SHR = mybir.AluOpType.logical_shift_right
    SHL = mybir.AluOpType.logical_shift_left
    OR = mybir.AluOpType.bitwise_or
    ADD = mybir.AluOpType.add

    with tile.TileContext(nc) as tc:
        with (
            tc.tile_pool(name="const", bufs=1) as constp,
            tc.tile_pool(name="persist", bufs=1) as persistp,
            tc.tile_pool(name="meta", bufs=2) as metap,
            tc.tile_pool(name="slab", bufs=2) as slabp,
            tc.tile_pool(name="p1", bufs=4) as pp,
            tc.tile_pool(name="sb", bufs=4) as sbp,
            tc.tile_pool(name="agg_ps", bufs=2, space="PSUM") as aggps,
            tc.tile_pool(name="mm_ps", bufs=2, space="PSUM") as mmps,
            tc.tile_pool(name="tp_ps", bufs=2, space="PSUM") as tpps,
            tc.tile_pool(name="dram", bufs=1, space="DRAM") as dramp,
        ):
            # on-device constants: iota row (512-wide) + bf16 identity
            iota_i = constp.tile([128, BW], i32)
            nc.gpsimd.iota(iota_i[:], pattern=[[1, BW]], base=0,
                           channel_multiplier=0)
            ones = constp.tile([128, 128], b16)
            nc.vector.memset(ones[:], 1.0)
            ident = constp.tile([128, 128], b16)
            nc.gpsimd.affine_select(out=ident[:], in_=ones[:],
                                    pattern=[[-1, 128]], compare_op=EQ,
                                    fill=0.0, base=0, channel_multiplier=1)

            # weight blob: each core ships 1/8th, AllGather reassembles
            wstage = dramp.tile([WTOT8], fp32, name="wstage")
            nc.sync.dma_start(wstage[:], wpart_in[:])
            wfull = dramp.tile([WPAD], fp32, addr_space="Shared", name="wfull")
            nc.gpsimd.collective_compute(
                "AllGather", mybir.AluOpType.bypass,
                replica_groups=[list(range(NCORES))],
                ins=[wstage[:].opt()], outs=[wfull[:].opt()])

            # weights from the blob: matmul weights cast to bf16 via SWDGE,
            # biases stay fp32
            wt = {}
            off = 0
            for name, k, m in WSPECS:
                ap = wfull[off:off + k * m].rearrange("(k m) -> k m", m=m)
                if m > 1:
                    t = constp.tile([k, m], b16, tag=f"w16_{name}",
                                    name=f"w16_{name}")
                    nc.gpsimd.dma_start(t[:], ap)
                else:
                    t = constp.tile([k, 1], fp32, tag=f"b_{name}",
                                    name=f"b_{name}")
                    nc.sync.dma_start(t[:], ap)
                wt[name] = t
                off += k * m
            gW = [wt["gW0"], wt["gW1"], wt["gW2"]]
            rW = [wt["rW0"], wt["rW1"], wt["rW2"]]
            gb = [wt["gb0"], wt["gb1"], wt["gb2"]]
            rb = [wt["rb0"], wt["rb1"], wt["rb2"]]
            cW1, cW2, cb1, cb2 = wt["cW1"], wt["cW2"], wt["cb1"], wt["cb2"]

            # DRAM scratch: full feature tables (AllGathered), local
            # transposed features, final local features
            table0 = dramp.tile([NTOT, IN_F], f8, addr_space="Shared",
                                name="table0")
            cc_in = [dramp.tile([NPAD, HID], b16, name=f"cc_in{i}") for i in range(2)]
            cc_out = [dramp.tile([NTOT, HID], b16, addr_space="Shared",
                                 name=f"cc_out{i}") for i in range(2)]
            hT_dram = dramp.tile([HID, NPAD], b16, name="hT_dram")
            h3_local = dramp.tile([NPAD, HID], b16, name="h3_local")

            h0_stage = dramp.tile([NPAD, IN_F], f8, name="h0_stage")
            nc.sync.dma_start(h0_stage[:], h0_in[:])
            nc.gpsimd.collective_compute(
                "AllGather", mybir.AluOpType.bypass,
                replica_groups=[list(range(NCORES))],
                ins=[h0_stage[:].opt()], outs=[table0[:].opt()])

            for L in range(3):
                DIN = IN_F if L == 0 else HID
                table_ap = table0[:] if L == 0 else cc_out[L - 1][:]
                with tc.For_i(0, NB2, 1) as g:
                    lo_t = metap.tile([128, COLS2], mybir.dt.uint16, tag="lo")
                    nc.sync.dma_start(lo_t[:], elo_ap[:, bass.ds(g, 1), :])
                    hi_t = metap.tile([128, COLS2 * 3 // 2], mybir.dt.uint8, tag="hi")
                    nc.sync.dma_start(hi_t[:], ehi_ap[:, bass.ds(g, 1), :])
                    # rebuild the 28-bit word: lo | a<<16 | (b&0xF)<<24 (even)
                    #                          lo | (b>>4)<<16 | c<<20   (odd)
                    lo32 = metap.tile([128, COLS2], i32, tag="lo32")
                    nc.vector.tensor_copy(lo32[:], lo_t[:])
                    hi32 = metap.tile([128, COLS2 * 3 // 2], i32, tag="hi32")
                    nc.vector.tensor_copy(hi32[:], hi_t[:])
                    hi3 = hi32[:].rearrange("p (j t) -> p t j", t=3)
                    a_ap, b_ap, c_ap = hi3[:, 0:1, :], hi3[:, 1:2, :], hi3[:, 2:3, :]
                    w_t = metap.tile([128, COLS2], i32, tag="w")
                    w2 = w_t[:].rearrange("p (j t) -> p t j", t=2)
                    lo2 = lo32[:].rearrange("p (j t) -> p t j", t=2)
                    NP = COLS2 // 2
                    t1 = metap.tile([128, NP], i32, tag="t1")
                    nc.vector.tensor_scalar(out=t1[:], in0=a_ap, scalar1=16,
                                            scalar2=None, op0=SHL)
                    t2 = metap.tile([128, NP], i32, tag="t2")
                    nc.vector.tensor_scalar(out=t2[:], in0=b_ap, scalar1=0xF,
                                            scalar2=24, op0=AND, op1=SHL)
                    nc.vector.tensor_tensor(out=t1[:], in0=t1[:],
                                            in1=t2[:], op=OR)
                    nc.vector.tensor_tensor(out=w2[:, 0:1, :], in0=t1[:],
                                            in1=lo2[:, 0:1, :], op=OR)
                    t3 = metap.tile([128, NP], i32, tag="t3")
                    nc.vector.tensor_scalar(out=t3[:], in0=b_ap, scalar1=4,
                                            scalar2=16, op0=SHR, op1=SHL)
                    t4 = metap.tile([128, NP], i32, tag="t4")
                    nc.vector.tensor_scalar(out=t4[:], in0=c_ap, scalar1=20,
                                            scalar2=None, op0=SHL)
                    nc.vector.tensor_tensor(out=t3[:], in0=t3[:],
                                            in1=t4[:], op=OR)
                    nc.vector.tensor_tensor(out=w2[:, 1:2, :], in0=t3[:],
                                            in1=lo2[:, 1:2, :], op=OR)
                    idx_t = metap.tile([128, COLS2], i32, tag="idx")
                    nc.vector.tensor_scalar(out=idx_t[:], in0=w_t[:],
                                            scalar1=0x7FFFF, scalar2=None,
                                            op0=AND)
                    lane_t = metap.tile([128, COLS2], i32, tag="lane")
                    nc.vector.tensor_scalar(out=lane_t[:], in0=w_t[:],
                                            scalar1=19, scalar2=None, op0=SHR)

                    if L == 0:
                        slab = slabp.tile([128, COLS2, IN_F], f8, tag="slab8",
                                          name="slab8")[:, :, :DIN]
                    else:
                        slab = slabp.tile([128, COLS2, IN_F], b16, tag="slab",
                                          name="slab")[:, :, :DIN]
                    for j in range(COLS2):
                        nc.gpsimd.indirect_dma_start(
                            out=slab[:, j, :], out_offset=None, in_=table_ap,
                            in_offset=bass.IndirectOffsetOnAxis(
                                ap=idx_t[:, j:j + 1], axis=0))

                    # residual rhs: transposed local features for this block
                    hTg = slabp.tile([IN_F, BW], b16, tag="hTg",
                                     name="hTg")[:DIN, :]
                    if L == 0:
                        h0blk8 = slabp.tile([128, GRP * IN_F], f8, tag="h0blk8")
                        for b6 in range(GRP):
                            nc.sync.dma_start(
                                h0blk8[:, b6 * IN_F:(b6 + 1) * IN_F],
                                h0r_ap[bass.ds(g, 1), b6:b6 + 1, :, :])
                        h0blk = slabp.tile([128, GRP * IN_F], b16, tag="h0blk")
                        nc.vector.tensor_copy(h0blk[:], h0blk8[:])
                        for b6 in range(GRP):
                            tp0 = tpps.tile([IN_F, 128], b16, space="PSUM",
                                            tag="tp")
                            nc.tensor.transpose(
                                out=tp0[:], in_=h0blk[:, b6 * IN_F:(b6 + 1) * IN_F],
                                identity=ident[:])
                            nc.vector.tensor_copy(
                                hTg[:, b6 * 128:(b6 + 1) * 128], tp0[:])
                    else:
                        nc.sync.dma_start(hTg[:], hT_dram[:, bass.ts(g, BW)])

                    # segment-sum via one-hot matmuls into one 512-wide psum
                    psum = aggps.tile([DIN, BW], fp32, space="PSUM", tag="agg")
                    for j in range(COLS2):
                        if L == 0:
                            p_t = pp.tile([128, BW], f8, tag="p8", name="p8")
                        else:
                            p_t = pp.tile([128, BW], b16, tag="p16", name="p16")
                        nc.vector.tensor_tensor(
                            out=p_t[:],
                            in0=lane_t[:, j:j + 1].to_broadcast([128, BW]),
                            in1=iota_i[:], op=EQ)
                        nc.tensor.matmul(out=psum[:], lhsT=slab[:, j, :],
                                         rhs=p_t[:], start=(j == 0),
                                         stop=(j == COLS2 - 1))
                    aggT = sbp.tile([DIN, BW], b16, tag="aggT")
                    nc.vector.tensor_copy(aggT[:], psum[:])
                    convp = mmps.tile([HID, BW], fp32, space="PSUM", tag="conv")
                    nc.tensor.matmul(out=convp[:], lhsT=gW[L][:], rhs=aggT[:],
                                     start=True, stop=True)
                    resp = mmps.tile([HID, BW], fp32, space="PSUM", tag="res")
                    nc.tensor.matmul(out=resp[:], lhsT=rW[L][:], rhs=hTg[:],
                                     start=True, stop=True)
                    convs = sbp.tile([HID, BW], b16, tag="convs")
                    nc.scalar.activation(convs[:], convp[:], Relu, bias=gb[L][:, :1])
                    ress = sbp.tile([HID, BW], b16, tag="ress")
                    nc.scalar.activation(ress[:], resp[:], Relu, bias=rb[L][:, :1])
                    hnewT = sbp.tile([HID, BW], b16, tag="hnewT")
                    nc.vector.tensor_add(hnewT[:], convs[:], ress[:])
                    if L < 2:
                        nc.sync.dma_start(hT_dram[:, bass.ts(g, BW)], hnewT[:])
                    dst_dram = cc_in[L] if L < 2 else h3_local
                    dd = dst_dram[:].rearrange("(g x) d -> g x d", x=BW)
                    for b6 in range(GRP):
                        tp = tpps.tile([128, HID], b16, space="PSUM", tag="tp")
                        nc.tensor.transpose(
                            out=tp[:], in_=hnewT[:, b6 * 128:(b6 + 1) * 128],
                            identity=ident[:HID, :HID])
                        hnew = sbp.tile([128, HID], b16, tag="hnew")
                        nc.vector.tensor_copy(hnew[:], tp[:])
                        nc.sync.dma_start(
                            dd[bass.ds(g, 1), b6 * 128:(b6 + 1) * 128, :], hnew[:])
                if L < 2:
                    nc.gpsimd.collective_compute(
                        "AllGather", mybir.AluOpType.bypass,
                        replica_groups=[list(range(NCORES))],
                        ins=[cc_in[L][:].opt()], outs=[cc_out[L][:].opt()])

            # -------- pooling + classifier --------
            out_slab = persistp.tile([N_CLASSES, GPAD], fp32)
            with tc.For_i(0, NGB, 1) as gbv:
                plo_t = metap.tile([128, PT], mybir.dt.uint16, tag="plo")
                nc.sync.dma_start(plo_t[:], plo_ap[:, bass.ds(gbv, 1), :])
                phi_t = metap.tile([128, PT], mybir.dt.uint8, tag="phi")
                nc.sync.dma_start(phi_t[:], phi_ap[:, bass.ds(gbv, 1), :])
                pidx_t = metap.tile([128, PT], i32, tag="pidx")
                nc.vector.tensor_copy(pidx_t[:], plo_t[:])
                plane_t = metap.tile([128, PT], i32, tag="plane")
                nc.vector.tensor_copy(plane_t[:], phi_t[:])
                pslab = slabp.tile([128, PT, HID], b16, tag="pslab")
                for t in range(PT):
                    nc.gpsimd.indirect_dma_start(
                        out=pslab[:, t, :], out_offset=None, in_=h3_local[:],
                        in_offset=bass.IndirectOffsetOnAxis(
                            ap=pidx_t[:, t:t + 1], axis=0))
                gpsum = aggps.tile([HID, 128], fp32, space="PSUM", tag="agg",
                                   name="gpsum")
                for t in range(PT):
                    p_t = pp.tile([128, 128], b16, tag="pp")
                    nc.vector.tensor_tensor(
                        out=p_t[:], in0=plane_t[:, t:t + 1].to_broadcast([128, 128]),
                        in1=iota_i[:, 0:128], op=EQ)
                    nc.tensor.matmul(out=gpsum[:], lhsT=pslab[:, t, :], rhs=p_t[:],
                                     start=(t == 0), stop=(t == PT - 1))
                graphT = sbp.tile([HID, 128], b16, tag="graphT")
                nc.vector.tensor_copy(graphT[:], gpsum[:])
                hidp = mmps.tile([CLS_H, 128], fp32, space="PSUM", tag="conv",
                                 name="hidp")
                nc.tensor.matmul(out=hidp[:], lhsT=cW1[:], rhs=graphT[:],
                                 start=True, stop=True)
                hid = sbp.tile([CLS_H, 128], b16, tag="hids")
                nc.scalar.activation(hid[:], hidp[:], Relu, bias=cb1[:, :1])
                logp = tpps.tile([N_CLASSES, 128], fp32, space="PSUM", tag="tp",
                                 name="logp")
                nc.tensor.matmul(out=logp[:], lhsT=cW2[:], rhs=hid[:],
                                 start=True, stop=True)
                nc.vector.tensor_tensor(
                    out=out_slab[:, bass.ts(gbv, 128)],
                    in0=cb2[:, 0:1].to_broadcast([N_CLASSES, 128]),
                    in1=logp[:], op=ADD)
            nc.sync.dma_start(logits_out[:], out_slab[:])

    nc.compile()
    return nc


def kernel(node_feats, src, dst, graph_ids,
           gW0, gb0, rW0, rb0, gW1, gb1, rW1, rb1, gW2, gb2, rW2, rb2,
           cW1, cb1, cW2, cb2):
    from concourse.bass_utils import run_bass_kernel_spmd

    node_feats = np.asarray(node_feats)
    cuts, gstart, per_core = _host_prep(node_feats, src, dst, graph_ids)

    if "nc" not in _cache:
        _cache["nc"] = _build_nc()
    nc = _cache["nc"]

    wvals = dict(
        gW0=gW0, rW0=rW0, gW1=gW1, rW1=rW1, gW2=gW2, rW2=rW2,
        cW1=cW1, cW2=cW2, gb0=gb0, rb0=rb0, gb1=gb1, rb1=rb1,
        gb2=gb2, rb2=rb2, cb1=cb1, cb2=cb2,
    )
    wblob = np.concatenate(
        [np.asarray(wvals[name], np.float32).reshape(-1) for name, _, _ in WSPECS]
        + [np.zeros(WPAD - WTOT, np.float32)])

    in_maps = [dict(h0=per_core[k]["h0"], lo16=per_core[k]["lo16"],
                    hi8=per_core[k]["hi8"],
                    wpart=wblob[k * WTOT8:(k + 1) * WTOT8])
               for k in range(NCORES)]

    import time as _time
    _t0 = _time.perf_counter()
    res = run_bass_kernel_spmd(nc, in_maps, core_ids=list(range(NCORES)))
    _cache["last_run_wall_s"] = _time.perf_counter() - _t0

    out = np.zeros((N_GRAPHS, N_CLASSES), np.float32)
    for k in range(NCORES):
        ng = gstart[k + 1] - gstart[k]
        out[gstart[k]:gstart[k + 1]] = res.results[k]["logitsT"][:, :ng].T
    return out


# revision 21
# speedup vs baseline: 1.0361x; 1.0361x over previous
"""DGL-life GCN classifier on 8 Trainium2 NeuronCores (Bass/Tile).

Strategy: shard the batched graph by dst-node across 8 cores (cuts aligned to
graph boundaries).  The axon tunnel to the devices moves ~45 MB/s, so the
wire format is minimized: each core receives only its local fp8 feature
shard (the full table is built on-device with an AllGather), edge metadata
ships 3.5 bytes per edge slot (uint16 low half + the 12 high bits of each
slot pair packed into 3 bytes, rebuilt on-device with bitvec DVE ops), and
the weights ride in a per-core 1/8th slice of one fp32 blob (AllGathered on
device).  Local nodes are permuted
(serpentine deal by in-degree) so each 512-dst block needs exactly 64 gather
columns.  Per layer: per-edge rows are fetched with indirect DMA gathers,
segment-summed into 512-dst blocks via one-hot matmuls accumulated in PSUM,
then the dense GraphConv / residual transforms run in the transposed domain
where the per-output-feature bias rides the ACT relu for free.  Feature
shards are exchanged between layers with an AllGather.  SumPooling reuses
the same one-hot matmul machinery over graph ids, followed by the 2-layer
MLP classifier.
"""
import sys
sys.path.insert(0, "/opt/trn_rl_repo")

import numpy as np
import ml_dtypes

bf16 = ml_dtypes.bfloat16
fp8 = ml_dtypes.float8_e4m3

N_NODES = 500000
N_EDGES = 8000000
N_GRAPHS = 16384
IN_F = 74
HID = 64
CLS_H = 128
N_CLASSES = 2

NCORES = 8
NPAD = 62976          # padded nodes per shard (123 blocks of 512)
NB2 = 123             # 512-node dst blocks per shard (one For_i group each)
BW = 512              # dst-block width (one-hot lane count)
COLS2 = 64            # gather columns per block (8192 edge slots, balanced)
GRP = 4               # 128-row sub-tiles per block (residual transposes)
NTOT = NCORES * NPAD  # 503808 padded global rows
GPAD = 2176           # padded graphs per shard (17 blocks of 128)
NGB = GPAD // 128     # 17
PT = 33               # pooling node tiles per graph block
ETOT = NB2 * COLS2    # edge-meta columns (7872)
PTOT = NGB * PT       # pool-meta columns (561)
WTOT8 = None          # set below: per-core slice of the padded weight blob
# invalid edge slots gather a guaranteed-pad table row (every 512-bin holds
# <=509 real nodes, so position 509 is pad everywhere) into a pad lane (511);
# pad lanes never feed real rows, so garbage stays quarantined.  This keeps
# the per-slot payload at 28 bits: uint16 low half + 12 high bits, with the
# high bits of each slot pair packed into 3 bytes.
INVALID_E = 509 | (511 << 19)
INVALID_P = 255 << 16  # pool lane 255 never matches iota 0..127
EHI = ETOT * 3 // 2    # packed high-byte columns for edge meta (11808)

# weight blob layout: (name, rows, cols); m>1 entries become bf16 tiles
WSPECS = [
    ("gW0", IN_F, HID), ("rW0", IN_F, HID),
    ("gW1", HID, HID), ("rW1", HID, HID),
    ("gW2", HID, HID), ("rW2", HID, HID),
    ("cW1", HID, CLS_H), ("cW2", CLS_H, N_CLASSES),
    ("gb0", HID, 1), ("rb0", HID, 1), ("gb1", HID, 1), ("rb1", HID, 1),
    ("gb2", HID, 1), ("rb2", HID, 1), ("cb1", CLS_H, 1), ("cb2", N_CLASSES, 1),
]
WTOT = sum(k * m for _, k, m in WSPECS)
WPAD = (WTOT + NCORES - 1) // NCORES * NCORES
WTOT8 = WPAD // NCORES

_cache = {}


def _host_prep(node_feats, src, dst, graph_ids):
    gid = np.asarray(graph_ids)
    node_of_graph_start = np.searchsorted(gid, np.arange(N_GRAPHS))
    cuts = [0]
    for k in range(1, NCORES):
        target = k * N_NODES // NCORES
        gi = np.searchsorted(node_of_graph_start, target)
        cand = [node_of_graph_start[min(gi, N_GRAPHS - 1)],
                node_of_graph_start[max(gi - 1, 0)]]
        cuts.append(int(min(cand, key=lambda x: abs(x - target))))
    cuts.append(N_NODES)
    cuts = np.asarray(cuts, np.int64)
    assert np.diff(cuts).max() <= NPAD

    gstart = [int(gid[c]) if c < N_NODES else N_GRAPHS for c in cuts[:-1]] + [N_GRAPHS]
    gstart = np.asarray(gstart, np.int64)
    assert np.diff(gstart).max() <= GPAD

    src = np.asarray(src).astype(np.int64)
    dst = np.asarray(dst).astype(np.int64)
    shard_of_dst = np.searchsorted(cuts, dst, side="right") - 1

    # pass 1: per-shard node permutation balancing edge load over 512-blocks
    # (serpentine deal of nodes sorted by in-degree)
    perms, masks = [], []
    perm_glob = np.empty(N_NODES, np.int64)
    for k in range(NCORES):
        base, n = cuts[k], cuts[k + 1] - cuts[k]
        m = shard_of_dst == k
        masks.append(m)
        indeg = np.bincount(dst[m] - base, minlength=n)
        order = np.argsort(-indeg, kind="stable")
        ids = np.arange(n)
        r, i = ids // NB2, ids % NB2
        bin_ = np.where(r % 2 == 0, i, NB2 - 1 - i)
        perm = np.empty(n, np.int64)
        perm[order] = bin_ * BW + r
        perms.append(perm)
        perm_glob[base:base + n] = k * NPAD + perm

    src_pad = perm_glob[src]

    per_core = []
    for k in range(NCORES):
        base, n = cuts[k], cuts[k + 1] - cuts[k]
        m, perm = masks[k], perms[k]
        e_src = src_pad[m]
        e_dst = perm[dst[m] - base]
        order = np.argsort(e_dst, kind="stable")
        e_src, e_dst = e_src[order], e_dst[order]
        blk = e_dst >> 9
        cnt = np.bincount(blk, minlength=NB2)
        assert cnt.max() <= COLS2 * 128, cnt.max()
        cum = np.concatenate([[0], np.cumsum(cnt)])
        slot = np.arange(len(e_dst)) - cum[blk]
        eword = np.full((NB2, COLS2 * 128), INVALID_E, np.int32)
        eword[blk, slot] = (e_src | ((e_dst & (BW - 1)) << 19)).astype(np.int32)
        eword = (eword.reshape(NB2, COLS2, 128)
                 .transpose(2, 0, 1).reshape(128, ETOT))
        elo = (eword & 0xFFFF).astype(np.uint16)
        hi12 = (eword >> 16).astype(np.int32)          # 12 bits per slot
        even, odd = hi12[:, 0::2], hi12[:, 1::2]
        ehi = np.stack([even & 0xFF,
                        (even >> 8) | ((odd & 0xF) << 4),
                        odd >> 4], axis=-1).astype(np.uint8).reshape(128, EHI)

        # pooling: local nodes sorted by graph; graph-block-aligned slots
        gl = gid[base:base + n] - gstart[k]          # local graph id per node
        gb = gl >> 7
        pcnt = np.bincount(gb, minlength=NGB)
        assert pcnt.max() <= PT * 128
        pcum = np.concatenate([[0], np.cumsum(pcnt)])
        pslot = np.arange(n) - pcum[gb]
        pword = np.full((NGB, PT * 128), INVALID_P, np.int32)
        pword[gb, pslot] = (perm | ((gl & 127) << 16)).astype(np.int32)
        pword = (pword.reshape(NGB, PT, 128)
                 .transpose(2, 0, 1).reshape(128, PTOT))
        plo = (pword & 0xFFFF).astype(np.uint16)
        phi = (pword >> 16).astype(np.uint8)

        lo16 = np.concatenate([elo, plo], axis=1)
        hi8 = np.concatenate([ehi, phi], axis=1)

        h0 = np.zeros((NPAD, IN_F), fp8)
        h0[perm] = node_feats[base:base + n].astype(fp8)

        per_core.append(dict(lo16=np.ascontiguousarray(lo16),
                             hi8=np.ascontiguousarray(hi8), h0=h0))
    return cuts, gstart, per_core


def _build_nc():
    import concourse.bass as bass
    from concourse import bacc
    import concourse.mybir as mybir
    import concourse.tile as tile

    fp32 = mybir.dt.float32
    b16 = mybir.dt.bfloat16
    f8 = mybir.dt.float8e4
    i32 = mybir.dt.int32

    nc = bacc.Bacc("TRN2", target_bir_lowering=False, debug=False,
                   num_devices=NCORES)

    h0_in = nc.dram_tensor("h0", [NPAD, IN_F], f8, kind="ExternalInput")
    lo16_in = nc.dram_tensor("lo16", [128, ETOT + PTOT], mybir.dt.uint16, kind="ExternalInput")
    hi8_in = nc.dram_tensor("hi8", [128, EHI + PTOT], mybir.dt.uint8, kind="ExternalInput")
    wpart_in = nc.dram_tensor("wpart", [WTOT8], fp32, kind="ExternalInput")
    logits_out = nc.dram_tensor("logitsT", [N_CLASSES, GPAD], fp32, kind="ExternalOutput")

    elo_ap = lo16_in[:, 0:ETOT].rearrange("p (g c) -> p g c", c=COLS2)
    ehi_ap = hi8_in[:, 0:EHI].rearrange("p (g c) -> p g c", c=COLS2 * 3 // 2)
    plo_ap = lo16_in[:, ETOT:ETOT + PTOT].rearrange("p (g c) -> p g c", c=PT)
    phi_ap = hi8_in[:, EHI:EHI + PTOT].rearrange("p (g c) -> p g c", c=PT)
    # L0 residual rhs source: [g, b, 128, f] view of the local shard
    h0r_ap = h0_in[:].rearrange("(g b p) f -> g b p f", b=GRP, p=128)

    Relu = mybir.ActivationFunctionType.Relu
    EQ = mybir.AluOpType.is_equal
    AND = mybir.AluOpType.bitwise_and
    SHR = mybir.AluOpType.logical_shift_right
    SHL = mybir.AluOpType.logical_shift_left
    OR = mybir.AluOpType.bitwise_or
    ADD = mybir.AluOpType.add

    with tile.TileContext(nc) as tc:
        with (
            tc.tile_pool(name="const", bufs=1) as constp,
            tc.tile_pool(name="persist", bufs=1) as persistp,
            tc.tile_pool(name="meta", bufs=2) as metap,
            tc.tile_pool(name="slab", bufs=2) as slabp,
            tc.tile_pool(name="p1", bufs=4) as pp,
            tc.tile_pool(name="sb", bufs=4) as sbp,
            tc.tile_pool(name="agg_ps", bufs=2, space="PSUM") as aggps,
            tc.tile_pool(name="mm_ps", bufs=2, space="PSUM") as mmps,
            tc.tile_pool(name="tp_ps", bufs=2, space="PSUM") as tpps,
            tc.tile_pool(name="dram", bufs=1, space="DRAM") as dramp,
        ):
            # on-device constants: iota row (512-wide) + bf16 identity
            iota_i = constp.tile([128, BW], i32)
            nc.gpsimd.iota(iota_i[:], pattern=[[1, BW]], base=0,
                           channel_multiplier=0)
            ones = constp.tile([128, 128], b16)
            nc.vector.memset(ones[:], 1.0)
            ident = constp.tile([128, 128], b16)
            nc.gpsimd.affine_select(out=ident[:], in_=ones[:],
                                    pattern=[[-1, 128]], compare_op=EQ,
                                    fill=0.0, base=0, channel_multiplier=1)

            # weight blob: each core ships 1/8th, AllGather reassembles
            wstage = dramp.tile([WTOT8], fp32, name="wstage")
            nc.sync.dma_start(wstage[:], wpart_in[:])
            wfull = dramp.tile([WPAD], fp32, addr_space="Shared", name="wfull")
            nc.gpsimd.collective_compute(
                "AllGather", mybir.AluOpType.bypass,
                replica_groups=[list(range(NCORES))],
                ins=[wstage[:].opt()], outs=[wfull[:].opt()])

            # weights from the blob: matmul weights cast to bf16 via SWDGE,
            # biases stay fp32
            wt = {}
            off = 0
            for name, k, m in WSPECS:
                ap = wfull[off:off + k * m].rearrange("(k m) -> k m", m=m)
                if m > 1:
                    t = constp.tile([k, m], b16, tag=f"w16_{name}",
                                    name=f"w16_{name}")
                    nc.gpsimd.dma_start(t[:], ap)
                else:
                    t = constp.tile([k, 1], fp32, tag=f"b_{name}",
                                    name=f"b_{name}")
                    nc.sync.dma_start(t[:], ap)
                wt[name] = t
                off += k * m
            gW = [wt["gW0"], wt["gW1"], wt["gW2"]]
            rW = [wt["rW0"], wt["rW1"], wt["rW2"]]
            gb = [wt["gb0"], wt["gb1"], wt["gb2"]]
            rb = [wt["rb0"], wt["rb1"], wt["rb2"]]
            cW1, cW2, cb1, cb2 = wt["cW1"], wt["cW2"], wt["cb1"], wt["cb2"]

            # DRAM scratch: full feature tables (AllGathered), local
            # transposed features, final local features
            table0 = dramp.tile([NTOT, IN_F], f8, addr_space="Shared",
                                name="table0")
            cc_in = [dramp.tile([NPAD, HID], b16, name=f"cc_in{i}") for i in range(2)]
            cc_out = [dramp.tile([NTOT, HID], b16, addr_space="Shared",
                                 name=f"cc_out{i}") for i in range(2)]
            hT_dram = dramp.tile([HID, NPAD], b16, name="hT_dram")
            h3_local = dramp.tile([NPAD, HID], b16, name="h3_local")

            h0_stage = dramp.tile([NPAD, IN_F], f8, name="h0_stage")
            nc.sync.dma_start(h0_stage[:], h0_in[:])
            nc.gpsimd.collective_compute(
                "AllGather", mybir.AluOpType.bypass,
                replica_groups=[list(range(NCORES))],
                ins=[h0_stage[:].opt()], outs=[table0[:].opt()])

            for L in range(3):
                DIN = IN_F if L == 0 else HID
                table_ap = table0[:] if L == 0 else cc_out[L - 1][:]
                with tc.For_i(0, NB2, 1) as g:
                    lo_t = metap.tile([128, COLS2], mybir.dt.uint16, tag="lo")
                    nc.sync.dma_start(lo_t[:], elo_ap[:, bass.ds(g, 1), :])
                    hi_t = metap.tile([128, COLS2 * 3 // 2], mybir.dt.uint8, tag="hi")
                    nc.sync.dma_start(hi_t[:], ehi_ap[:, bass.ds(g, 1), :])
                    # rebuild the 28-bit word: lo | a<<16 | (b&0xF)<<24 (even)
                    #                          lo | (b>>4)<<16 | c<<20   (odd)
                    lo32 = metap.tile([128, COLS2], i32, tag="lo32")
                    nc.vector.tensor_copy(lo32[:], lo_t[:])
                    hi32 = metap.tile([128, COLS2 * 3 // 2], i32, tag="hi32")
                    nc.vector.tensor_copy(hi32[:], hi_t[:])
                    hi3 = hi32[:].rearrange("p (j t) -> p t j", t=3)
                    a_ap, b_ap, c_ap = hi3[:, 0:1, :], hi3[:, 1:2, :], hi3[:, 2:3, :]
                    w_t = metap.tile([128, COLS2], i32, tag="w")
                    w2 = w_t[:].rearrange("p (j t) -> p t j", t=2)
                    lo2 = lo32[:].rearrange("p (j t) -> p t j", t=2)
                    NP = COLS2 // 2
                    t1 = metap.tile([128, NP], i32, tag="t1")
                    nc.vector.tensor_scalar(out=t1[:], in0=a_ap, scalar1=16,
                                            scalar2=None, op0=SHL)
                    t2 = metap.tile([128, NP], i32, tag="t2")
                    nc.vector.tensor_scalar(out=t2[:], in0=b_ap, scalar1=0xF,
                                            scalar2=24, op0=AND, op1=SHL)
                    nc.vector.tensor_tensor(out=t1[:], in0=t1[:],
                                            in1=t2[:], op=OR)
                    nc.vector.tensor_tensor(out=w2[:, 0:1, :], in0=t1[:],
                                            in1=lo2[:, 0:1, :], op=OR)
                    t3 = metap.tile([128, NP], i32, tag="t3")
                    nc.vector.tensor_scalar(out=t3[:], in0=b_ap, scalar1=4,
                                            scalar2=16, op0=SHR, op1=SHL)
                    t4 = metap.tile([128, NP], i32, tag="t4")
                    nc.vector.tensor_scalar(out=t4[:], in0=c_ap, scalar1=20,
                                            scalar2=None, op0=SHL)
                    nc.vector.tensor_tensor(out=t3[:], in0=t3[:],
                                            in1=t4[:], op=OR)
                    nc.vector.tensor_tensor(out=w2[:, 1:2, :], in0=t3[:],
                                            in1=lo2[:, 1:2, :], op=OR)
                    idx_t = metap.tile([128, COLS2], i32, tag="idx")
                    nc.vector.tensor_scalar(out=idx_t[:], in0=w_t[:],
                                            scalar1=0x7FFFF, scalar2=None,
                                            op0=AND)
                    lane_t = metap.tile([128, COLS2], i32, tag="lane")
                    nc.vector.tensor_scalar(out=lane_t[:], in0=w_t[:],
                                            scalar1=19, scalar2=None, op0=SHR)

                    if L == 0:
                        slab = slabp.tile([128, COLS2, IN_F], f8, tag="slab8",
                                          name="slab8")[:, :, :DIN]
                    else:
                        slab = slabp.tile([128, COLS2, IN_F], b16, tag="slab",
                                          name="slab")[:, :, :DIN]
                    for j in range(COLS2):
                        nc.gpsimd.indirect_dma_start(
                            out=slab[:, j, :], out_offset=None, in_=table_ap,
                            in_offset=bass.IndirectOffsetOnAxis(
                                ap=idx_t[:, j:j + 1], axis=0))

                    # residual rhs: transposed local features for this block
                    hTg = slabp.tile([IN_F, BW], b16, tag="hTg",
                                     name="hTg")[:DIN, :]
                    if L == 0:
                        h0blk8 = slabp.tile([128, GRP * IN_F], f8, tag="h0blk8")
                        for b6 in range(GRP):
                            nc.sync.dma_start(
                                h0blk8[:, b6 * IN_F:(b6 + 1) * IN_F],
                                h0r_ap[bass.ds(g, 1), b6:b6 + 1, :, :])
                        h0blk = slabp.tile([128, GRP * IN_F], b16, tag="h0blk")
                        nc.vector.tensor_copy(h0blk[:], h0blk8[:])
                        for b6 in range(GRP):
                            tp0 = tpps.tile([IN_F, 128], b16, space="PSUM",
                                            tag="tp")
                            nc.tensor.transpose(
                                out=tp0[:], in_=h0blk[:, b6 * IN_F:(b6 + 1) * IN_F],
                                identity=ident[:])
                            nc.vector.tensor_copy(
                                hTg[:, b6 * 128:(b6 + 1) * 128], tp0[:])
                    else:
                        nc.sync.dma_start(hTg[:], hT_dram[:, bass.ts(g, BW)])

                    # segment-sum via one-hot matmuls into one 512-wide psum
                    psum = aggps.tile([DIN, BW], fp32, space="PSUM", tag="agg")
                    for j in range(COLS2):
                        if L == 0:
                            p_t = pp.tile([128, BW], f8, tag="p8", name="p8")
                        else:
                            p_t = pp.tile([128, BW], b16, tag="p16", name="p16")
                        nc.vector.tensor_tensor(
                            out=p_t[:],
                            in0=lane_t[:, j:j + 1].to_broadcast([128, BW]),
                            in1=iota_i[:], op=EQ)
                        nc.tensor.matmul(out=psum[:], lhsT=slab[:, j, :],
                                         rhs=p_t[:], start=(j == 0),
                                         stop=(j == COLS2 - 1))
                    aggT = sbp.tile([DIN, BW], b16, tag="aggT")
                    nc.vector.tensor_copy(aggT[:], psum[:])
                    convp = mmps.tile([HID, BW], fp32, space="PSUM", tag="conv")
                    nc.tensor.matmul(out=convp[:], lhsT=gW[L][:], rhs=aggT[:],
                                     start=True, stop=True)
                    resp = mmps.tile([HID, BW], fp32, space="PSUM", tag="res")
                    nc.tensor.matmul(out=resp[:], lhsT=rW[L][:], rhs=hTg[:],
                                     start=True, stop=True)
                    convs = sbp.tile([HID, BW], b16, tag="convs")
                    nc.scalar.activation(convs[:], convp[:], Relu, bias=gb[L][:, :1])
                    ress = sbp.tile([HID, BW], b16, tag="ress")
                    nc.scalar.activation(ress[:], resp[:], Relu, bias=rb[L][:, :1])
                    hnewT = sbp.tile([HID, BW], b16, tag="hnewT")
                    nc.vector.tensor_add(hnewT[:], convs[:], ress[:])
                    if L < 2:
                        nc.sync.dma_start(hT_dram[:, bass.ts(g, BW)], hnewT[:])
                    dst_dram = cc_in[L] if L < 2 else h3_local
                    dd = dst_dram[:].rearrange("(g x) d -> g x d", x=BW)
                    for b6 in range(GRP):
                        tp = tpps.tile([128, HID], b16, space="PSUM", tag="tp")
                        nc.tensor.transpose(
                            out=tp[:], in_=hnewT[:, b6 * 128:(b6 + 1) * 128],
                            identity=ident[:HID, :HID])
                        hnew = sbp.tile([128, HID], b16, tag="hnew")
                        nc.vector.tensor_copy(hnew[:], tp[:])
                        nc.sync.dma_start(
                            dd[bass.ds(g, 1), b6 * 128:(b6 + 1) * 128, :], hnew[:])
                if L < 2:
                    nc.gpsimd.collective_compute(
                        "AllGather", mybir.AluOpType.bypass,
                        replica_groups=[list(range(NCORES))],
                        ins=[cc_in[L][:].opt()], outs=[cc_out[L][:].opt()])

            # -------- pooling + classifier --------
            out_slab = persistp.tile([N_CLASSES, GPAD], fp32)
            with tc.For_i(0, NGB, 1) as gbv:
                plo_t = metap.tile([128, PT], mybir.dt.uint16, tag="plo")
                nc.sync.dma_start(plo_t[:], plo_ap[:, bass.ds(gbv, 1), :])
                phi_t = metap.tile([128, PT], mybir.dt.uint8, tag="phi")
                nc.sync.dma_start(phi_t[:], phi_ap[:, bass.ds(gbv, 1), :])
                pidx_t = metap.tile([128, PT], i32, tag="pidx")
                nc.vector.tensor_copy(pidx_t[:], plo_t[:])
                plane_t = metap.tile([128, PT], i32, tag="plane")
                nc.vector.tensor_copy(plane_t[:], phi_t[:])
                pslab = slabp.tile([128, PT, HID], b16, tag="pslab")
                for t in range(PT):
                    nc.gpsimd.indirect_dma_start(
                        out=pslab[:, t, :], out_offset=None, in_=h3_local[:],
                        in_offset=bass.IndirectOffsetOnAxis(
                            ap=pidx_t[:, t:t + 1], axis=0))
                gpsum = aggps.tile([HID, 128], fp32, space="PSUM", tag="agg",
                                   name="gpsum")
                for t in range(PT):
                    p_t = pp.tile([128, 128], b16, tag="pp")
                    nc.vector.tensor_tensor(
                        out=p_t[:], in0=plane_t[:, t:t + 1].to_broadcast([128, 128]),
                        in1=iota_i[:, 0:128], op=EQ)
                    nc.tensor.matmul(out=gpsum[:], lhsT=pslab[:, t, :], rhs=p_t[:],
                                     start=(t == 0), stop=(t == PT - 1))
                graphT = sbp.tile([HID, 128], b16, tag="graphT")
                nc.vector.tensor_copy(graphT[:], gpsum[:])
                hidp = mmps.tile([CLS_H, 128], fp32, space="PSUM", tag="conv",
                                 name="hidp")
                nc.tensor.matmul(out=hidp[:], lhsT=cW1[:], rhs=graphT[:],
                                 start=True, stop=True)
                hid = sbp.tile([CLS_H, 128], b16, tag="hids")
                nc.scalar.activation(hid[:], hidp[:], Relu, bias=cb1[:, :1])
                logp = tpps.tile([N_CLASSES, 128], fp32, space="PSUM", tag="tp",
                                 name="logp")
                nc.tensor.matmul(out=logp[:], lhsT=cW2[:], rhs=hid[:],
                                 start=True, stop=True)
                nc.vector.tensor_tensor(
                    out=out_slab[:, bass.ts(gbv, 128)],
                    in0=cb2[:, 0:1].to_broadcast([N_CLASSES, 128]),
                    in1=logp[:], op=ADD)
            nc.sync.dma_start(logits_out[:], out_slab[:])

    nc.compile()
    return nc


def kernel(node_feats, src, dst, graph_ids,
           gW0, gb0, rW0, rb0, gW1, gb1, rW1, rb1, gW2, gb2, rW2, rb2,
           cW1, cb1, cW2, cb2):
    from concourse.bass_utils import run_bass_kernel_spmd

    node_feats = np.asarray(node_feats)
    cuts, gstart, per_core = _host_prep(node_feats, src, dst, graph_ids)

    if "nc" not in _cache:
        _cache["nc"] = _build_nc()
    nc = _cache["nc"]

    wvals = dict(
        gW0=gW0, rW0=rW0, gW1=gW1, rW1=rW1, gW2=gW2, rW2=rW2,
        cW1=cW1, cW2=cW2, gb0=gb0, rb0=rb0, gb1=gb1, rb1=rb1,
        gb2=gb2, rb2=rb2, cb1=cb1, cb2=cb2,
    )
    wblob = np.concatenate(
        [np.asarray(wvals[name], np.float32).reshape(-1) for name, _, _ in WSPECS]
        + [np.zeros(WPAD - WTOT, np.float32)])

    in_maps = [dict(h0=per_core[k]["h0"], lo16=per_core[k]["lo16"],
                    hi8=per_core[k]["hi8"],
                    wpart=wblob[k * WTOT8:(k + 1) * WTOT8])
               for k in range(NCORES)]

    import time as _time
    _t0 = _time.perf_counter()
    res = run_bass_kernel_spmd(nc, in_maps, core_ids=list(range(NCORES)))
    _cache["last_run_wall_s"] = _time.perf_counter() - _t0

    out = np.zeros((N_GRAPHS, N_CLASSES), np.float32)
    for k in range(NCORES):
        ng = gstart[k + 1] - gstart[k]
        out[gstart[k]:gstart[k + 1]] = res.results[k]["logitsT"][:, :ng].T
    return out
